# revision 27
# baseline (speedup 1.0000x reference)
"""Trainium2 Bass kernel for the FFT-block (attention + conv FFN) problem.

Sharding: data-parallel over batch. B=16 items across 8 cores -> 2 items/core.

v3: the conv FFN (85% of FLOPs) is computed via length-16 overlap-save
cyclic convolution in the real-DFT domain (fp16 transform-domain matmuls,
fp32 PSUM accumulation):
  - conv1 uses Karatsuba 3-mult complex products -> 23 mult-slots per 8
    outputs (vs 72 dense matmul slots); the Karatsuba recombination is
    folded into the inverse-transform (A) matrix.
  - conv2 uses plain 4-mult products -> 30 slots, which keeps the V2
    tensor small enough to hold both items in SBUF so U2 streams once.
  - B-stage (forward transform) / A-stage (inverse transform) are matmuls
    against constant matrices; conv biases fold in as K=1 matmuls on the
    DC slot.
  - M tensors are regrouped tau-major -> t-major with SBUF->SBUF DMAs so
    the A-stage contraction (over transform slots) runs on the PE.
DFT transforms are orthogonal, so fp16 transform-domain numerics land at
~6e-4 max-rel (vs 2.1e-3 for the dense bf16 v2 kernel).
"""
import sys, types
import numpy as np

B, S, D = 16, 1024, 512
H, DK = 8, 64
CD, KS = 2048, 9
EPS = 1e-5
NCORES = 8
NIT = B // NCORES
NDC = D // 128             # 4 d-chunks
NSC = S // 128             # 8 s-chunks
NCOL = S // 512            # 2 s-cols
NCD = CD // 128            # 16 cd-chunks

N16, M8 = 16, 8            # DFT length / outputs per tile
NT1 = 23                   # conv1 slots (Karatsuba)
NT2 = 16                   # conv2 V/M rows (plain)
NSL2 = 30                  # conv2 mult slots
U1B = 16 * NT1             # 368  B1 main cols
U2B = 16 * NT2             # 256  B2 main cols


def _install_ntff_hook():
    try:
        from antenv.axon_hooks import get_axon_ntff_profile_hook  # noqa
        return
    except ImportError:
        pass
    try:
        from trn_agent_boot.trn_boot import _ntff_profile_via_ctypes
        mod = types.ModuleType('antenv.axon_hooks')
        hook = _ntff_profile_via_ctypes('/opt/axon/libaxon_pjrt.so')
        mod.get_axon_ntff_profile_hook = lambda: hook
        sys.modules['antenv.axon_hooks'] = mod
    except Exception:
        pass


# ---------------- DFT-domain transform matrices (host, float64) ---------
def _dft_mats():
    """Returns BT1 [23,16], AT1 [8,23], BT2 [16,16], AT2 [8,16], SL2."""
    j = np.arange(N16)
    BT1 = np.zeros((NT1, N16))
    BT2 = np.zeros((NT2, N16))
    cr = lambda f: np.cos(2 * np.pi * j * f / N16)
    si = lambda f: -np.sin(2 * np.pi * j * f / N16)  # Im of e^{-2pi i jf/16}
    BT1[0] = cr(0); BT1[1] = cr(8)
    BT2[0] = cr(0); BT2[1] = cr(8)
    for f in range(1, 8):
        b1 = 2 + 3 * (f - 1)
        BT1[b1 + 0] = cr(f)
        BT1[b1 + 1] = si(f)
        BT1[b1 + 2] = cr(f) + si(f)
        b2 = 2 + 2 * (f - 1)
        BT2[b2 + 0] = cr(f)
        BT2[b2 + 1] = si(f)
    AT1 = np.zeros((M8, NT1))
    AT2 = np.zeros((M8, NT2))
    for ss in range(M8):
        AT1[ss, 0] = 1.0; AT1[ss, 1] = (-1.0) ** ss
        AT2[ss, 0] = 1.0; AT2[ss, 1] = (-1.0) ** ss
        for f in range(1, 8):
            th = 2 * np.pi * ss * f / N16
            co, sn = np.cos(th), np.sin(th)
            b1 = 2 + 3 * (f - 1)
            AT1[ss, b1 + 0] = 2 * (co + sn)
            AT1[ss, b1 + 1] = 2 * (-co + sn)
            AT1[ss, b1 + 2] = -2 * sn
            b2 = 2 + 2 * (f - 1)
            AT2[ss, b2 + 0] = 2 * co
            AT2[ss, b2 + 1] = -2 * sn
    # conv2 slot list: (bin f, part, sign, vrow, mrow), m-major order
    SL2 = [(0, 'r', 1.0, 0, 0), (8, 'r', 1.0, 1, 1)]
    for f in range(1, 8):
        b2 = 2 + 2 * (f - 1)
        SL2 += [(f, 'r', 1.0, b2 + 0, b2 + 0),     # Mr += Ur*Vr
                (f, 'i', -1.0, b2 + 1, b2 + 0),    # Mr -= Ui*Vi
                (f, 'r', 1.0, b2 + 1, b2 + 1),     # Mi += Ur*Vi
                (f, 'i', 1.0, b2 + 0, b2 + 1)]     # Mi += Ui*Vr
    return BT1, AT1, BT2, AT2, SL2


def _u_slots1(W):
    """W [CD, D, 9] -> U1 [23, D, CD] (1/16 folded)."""
    gp = np.zeros(W.shape[:2] + (N16,))
    gp[:, :, :KS] = W
    Gh = np.conj(np.fft.fft(gp, axis=2)) / N16      # [co, ci, 16]
    Ut = Gh.transpose(2, 1, 0)                      # [16, ci, co]
    U = np.zeros((NT1,) + Ut.shape[1:])
    U[0] = Ut[0].real; U[1] = Ut[8].real
    for f in range(1, 8):
        b1 = 2 + 3 * (f - 1)
        U[b1 + 0] = Ut[f].real
        U[b1 + 1] = Ut[f].imag
        U[b1 + 2] = Ut[f].real + Ut[f].imag
    return U


def _u_slots2(W, SL2):
    """W [D, CD, 9] -> U2 [30, CD, D] with signs and 1/16 folded."""
    gp = np.zeros(W.shape[:2] + (N16,))
    gp[:, :, :KS] = W
    Gh = np.conj(np.fft.fft(gp, axis=2)) / N16      # [co, ci, 16]
    Ut = Gh.transpose(2, 1, 0)                      # [16, ci, co]
    U = np.zeros((NSL2,) + Ut.shape[1:])
    for i, (f, part, sign, vrow, mrow) in enumerate(SL2):
        U[i] = sign * (Ut[f].real if part == 'r' else Ut[f].imag)
    return U


def _bmat(BT):
    """BT [nt, 16] -> Bmat [128, nt*16 + 2*nt] (main|prev|next cols)."""
    nt = BT.shape[0]
    nb = nt * 16
    Bm = np.zeros((128, nb + 2 * nt))
    for tl in range(16):
        for t in range(nt):
            for jj in range(N16):
                ss = 8 * tl + jj - 4
                if 0 <= ss < 128:
                    Bm[ss, t * 16 + tl] = BT[t, jj]
    for t in range(nt):
        for jj in range(4):                 # prev: rows 124..127 <- j 0..3
            Bm[124 + jj, nb + t] = BT[t, jj]
        for ss in range(4):                 # next: rows 0..3 <- j 12..15
            Bm[ss, nb + nt + t] = BT[t, 12 + ss]
    return Bm


def _amat_packed(AT, tlist):
    """A[i*8+tau, tau2*8+s] = AT[s, tlist[i]] * (tau==tau2) -> [n*8, 64]."""
    n = len(tlist)
    A = np.zeros((n * 8, 64))
    for i, t in enumerate(tlist):
        for tau in range(8):
            for ss in range(M8):
                A[i * 8 + tau, tau * M8 + ss] = AT[ss, t]
    return A


_BUILT = {}


def _build(affine1=False, affine2=False):
    """affine1/affine2: apply LN1/LN2 gamma,beta (skipped when g==1, b==0)."""
    global _BUILT
    key = (affine1, affine2)
    if key in _BUILT:
        return _BUILT[key]
    _install_ntff_hook()
    import concourse.bacc as bacc
    import concourse.mybir as mybir
    from concourse import tile
    from concourse.masks import make_identity
    from contextlib import ExitStack

    F32 = mybir.dt.float32
    F32R = mybir.dt.float32r
    BF16 = mybir.dt.bfloat16
    FP16 = mybir.dt.float16
    AF = mybir.ActivationFunctionType
    ALU = mybir.AluOpType

    _, _, _, _, SL2 = _dft_mats()

    nc = bacc.Bacc("TRN2", target_bir_lowering=False, debug=False,
                   num_devices=NCORES)

    # ---- DRAM I/O (per core) ----
    d_xT = nc.dram_tensor("xT", [NIT, NDC, 128, S], F32, kind="ExternalInput")
    d_xp = nc.dram_tensor("xp", [NIT, NSC, 128, D], F32, kind="ExternalInput")
    d_wqk = nc.dram_tensor("wqk", [2, 4, 128, 512], F32, kind="ExternalInput")
    d_bqk = nc.dram_tensor("bqk", [128, 8], F32, kind="ExternalInput")
    d_wv = nc.dram_tensor("wv", [NDC, 128, 520], F32, kind="ExternalInput")
    d_bvrow = nc.dram_tensor("bvrow", [128, 520], F32, kind="ExternalInput")
    d_wo = nc.dram_tensor("wo", [4, 128, 512], F32, kind="ExternalInput")
    d_gb = nc.dram_tensor("gb", [4, 128, 512], F32, kind="ExternalInput")
    d_cones = nc.dram_tensor("cones", [128, 128], F32, kind="ExternalInput")
    d_bm1 = nc.dram_tensor("bm1", [128, U1B + 2 * NT1], FP16,
                           kind="ExternalInput")
    d_bm2 = nc.dram_tensor("bm2", [128, U2B + 2 * NT2], FP16,
                           kind="ExternalInput")
    d_a0 = nc.dram_tensor("a0", [96, 64], FP16, kind="ExternalInput")
    d_a1 = nc.dram_tensor("a1", [88, 64], FP16, kind="ExternalInput")
    d_a2 = nc.dram_tensor("a2", [128, 64], FP16, kind="ExternalInput")
    d_ones = nc.dram_tensor("ones", [1, 128], FP16, kind="ExternalInput")
    d_bc1r = nc.dram_tensor("bc1r", [1, CD], FP16, kind="ExternalInput")
    d_bc2r = nc.dram_tensor("bc2r", [1, D], FP16, kind="ExternalInput")
    d_u1 = nc.dram_tensor("u1", [4, NT1, 128, NDC * 512], FP16,
                          kind="ExternalInput")
    d_u2 = nc.dram_tensor("u2", [NSL2, 4, 128, 4 * 512], FP16,
                          kind="ExternalInput")
    d_y = nc.dram_tensor("y", [NIT, NSC, 128, D], F32, kind="ExternalOutput")

    G1, B1b, G2, B2b = range(4)

    with tile.TileContext(nc) as tc:
        est = ExitStack()
        with est:
            cp = est.enter_context(tc.tile_pool(name="const", bufs=1))
            kp = est.enter_context(tc.tile_pool(name="keep", bufs=1))
            dp = est.enter_context(tc.tile_pool(name="dramp", bufs=1,
                                                space="DRAM"))

            # c1 spill (DRAM scratch), per (item, s-chunk)
            c1_dram = [[dp.tile([128, CD], FP16, tag=f"c1d{it}_{sc}",
                                name=f"c1d{it}_{sc}")
                        for sc in range(NSC)] for it in range(NIT)]

            # ---- persistent constants ----
            t_eps = cp.tile([128, 1], F32, tag="eps")
            nc.vector.memset(t_eps[:], EPS)
            t_gb = []
            for i in range(4):
                t = cp.tile([128, 512], F32, tag=f"gb{i}", name=f"gb{i}")
                if affine1 or affine2:
                    nc.sync.dma_start(t[:], d_gb[i])
                t_gb.append(t)
            t_bm1 = cp.tile([128, U1B + 2 * NT1], FP16, tag="bm1")
            nc.sync.dma_start(t_bm1[:], d_bm1[:])
            t_bm2 = cp.tile([128, U2B + 2 * NT2], FP16, tag="bm2")
            nc.sync.dma_start(t_bm2[:], d_bm2[:])
            t_a0 = cp.tile([96, 64], FP16, tag="a0")
            nc.sync.dma_start(t_a0[:], d_a0[:])
            t_a1 = cp.tile([88, 64], FP16, tag="a1")
            nc.sync.dma_start(t_a1[:], d_a1[:])
            t_a2 = cp.tile([128, 64], FP16, tag="a2")
            nc.sync.dma_start(t_a2[:], d_a2[:])
            t_ones = cp.tile([1, 128], FP16, tag="onesr")
            nc.sync.dma_start(t_ones[:], d_ones[:])
            t_bc1r = cp.tile([1, CD], FP16, tag="bc1r")
            nc.sync.dma_start(t_bc1r[:], d_bc1r[:])
            t_bc2r = cp.tile([1, D], FP16, tag="bc2r")
            nc.sync.dma_start(t_bc2r[:], d_bc2r[:])

            # LN1 output in fp16, persistent across phases
            h16 = [[kp.tile([128, D], FP16, tag=f"h16_{it}_{sc}",
                            name=f"h16_{it}_{sc}")
                    for sc in range(NSC)] for it in range(NIT)]

            state = [dict() for _ in range(NIT)]

            # =============== attention phase ===============
            with tc.tile_pool(name="attn", bufs=1) as ap, \
                 tc.tile_pool(name="psA", bufs=1, space="PSUM") as ps:

                def emit_x(it):
                    st = state[it]
                    xt = []
                    for dc in range(NDC):
                        t = ap.tile([128, S], F32R, tag=f"xt{dc}",
                                    name=f"xt{dc}")
                        nc.sync.dma_start(t[:], d_xT[it, dc].bitcast(F32R))
                        xt.append(t)
                    st["xt"] = xt
                    st["qkt"] = {}

                emit_x(0)

                t_wv = []
                for dc in range(NDC):
                    t = ap.tile([128, 520], F32R, tag=f"wv{dc}",
                                name=f"wv{dc}")
                    nc.sync.dma_start(t[:], d_wv[dc].bitcast(F32R))
                    t_wv.append(t)
                t_bvfull = ap.tile([128, 520], F32, tag="bvfull")
                nc.sync.dma_start(t_bvfull[:], d_bvrow[:])
                t_bqk = ap.tile([128, 8], F32, tag="bqk")
                nc.sync.dma_start(t_bqk[:], d_bqk[:])
                t_cones = ap.tile([128, 128], F32R, tag="cones")
                nc.sync.dma_start(t_cones[:], d_cones[:].bitcast(F32R))
                t_wo = []
                for c in range(4):
                    t = ap.tile([128, 512], F32R, tag=f"wo{c}", name=f"wo{c}")
                    nc.sync.dma_start(t[:], d_wo[c].bitcast(F32R))
                    t_wo.append(t)

                def emit_v(it):
                    st = state[it]
                    xt = st["xt"]
                    vst = []
                    for tc_i in range(NSC):
                        vt = ap.tile([128, 520], BF16, tag=f"vst{tc_i}",
                                     name=f"vst{tc_i}")
                        for half in range(2):
                            colo = half * 260
                            pv = ps.tile([128, 260], F32, tag="pp", bufs=2)
                            for dc in range(NDC):
                                nc.tensor.matmul(
                                    pv[:],
                                    xt[dc][:, tc_i * 128:(tc_i + 1) * 128],
                                    t_wv[dc][:, colo:colo + 260],
                                    start=(dc == 0), stop=(dc == NDC - 1))
                            nc.vector.tensor_tensor(
                                vt[:, colo:colo + 260], pv[:],
                                t_bvfull[:, colo:colo + 260], ALU.add)
                        vst.append(vt)
                    st["vst"] = vst

                def emit_qk(it, pair):
                    st = state[it]
                    xt = st["xt"]
                    for proj in range(2):
                        wt = ap.tile([128, 512], F32R, tag=f"wqk{proj}",
                                     bufs=2, name="wt")
                        nc.sync.dma_start(wt[:],
                                          d_wqk[proj, pair].bitcast(F32R))
                        qt = ap.tile([128, S], BF16, tag=f"qk{proj}{pair}",
                                     name="qt")
                        for scol in range(NCOL):
                            pq = ps.tile([128, 512], F32, tag="pp", bufs=2)
                            for dc in range(NDC):
                                nc.tensor.matmul(
                                    pq[:], wt[:, dc * 128:(dc + 1) * 128],
                                    xt[dc][:, scol * 512:(scol + 1) * 512],
                                    start=(dc == 0), stop=(dc == NDC - 1))
                            nc.vector.tensor_scalar_add(
                                qt[:, scol * 512:(scol + 1) * 512], pq[:],
                                t_bqk[:, proj * 4 + pair:proj * 4 + pair + 1])
                        st["qkt"][(proj, pair)] = qt

                def heads_gen(it):
                    st = state[it]
                    st["ctxT"] = [ap.tile([128, S], F32R, tag=f"ct{c}",
                                          name=f"ct{c}") for c in range(4)]
                    ctxT = st["ctxT"]
                    vst = st["vst"]
                    groups = [(p, s) for p in range(4) for s in range(2)]
                    pex = {}
                    pc = {}

                    def scores_slot(g, ti):
                        pair, scol = g
                        so = scol * 512
                        qT = st["qkt"][(0, pair)]
                        kT = st["qkt"][(1, pair)]
                        sc2 = ps.tile([128, 1024], F32, tag="sc2", bufs=2,
                                      name="sc2")
                        for sub in range(2):
                            hr = slice(sub * 64, sub * 64 + 64)
                            nc.tensor.matmul(
                                sc2[:, sub * 512:(sub + 1) * 512],
                                kT[hr, ti * 128:(ti + 1) * 128],
                                qT[hr, so:so + 512], start=True, stop=True)
                        pe = ap.tile([128, 1024], BF16, tag=f"pex{ti}",
                                     bufs=1, name="pe")
                        nc.scalar.activation(pe[:], sc2[:], AF.Exp,
                                             scale=0.125)
                        pex[ti] = pe

                    def ctx_slot(g, ti):
                        pair, scol = g
                        if ti == 0:
                            pc[g] = [ps.tile([65, 512], F32, tag="pc",
                                             bufs=2, name=f"pcx{sub}")
                                     for sub in range(2)]
                        for sub in range(2):
                            h = 2 * pair + sub
                            nc.tensor.matmul(
                                pc[g][sub][:], vst[ti][:, h * 65:h * 65 + 65],
                                pex[ti][:, sub * 512:(sub + 1) * 512],
                                start=(ti == 0), stop=(ti == NSC - 1))

                    def norm(g):
                        pair, scol = g
                        so = scol * 512
                        for sub in range(2):
                            hr = slice(sub * 64, sub * 64 + 64)
                            zr = ap.tile([1, 512], F32R, tag="bcs", bufs=4,
                                         name="zr")
                            nc.vector.tensor_copy(zr[0:1, :],
                                                  pc[g][sub][64:65, :])
                            pb = ps.tile([64, 512], F32, tag="pp", bufs=2,
                                         name="pb")
                            nc.tensor.matmul(pb[:], t_cones[0:1, 0:64],
                                             zr[0:1, :], start=True,
                                             stop=True)
                            bcs = ap.tile([64, 512], F32, tag="bcs2", bufs=4,
                                          name="bcs")
                            nc.vector.reciprocal_approx_fast(out=bcs[:],
                                                             in_=pb[:])
                            nc.vector.tensor_tensor(
                                ctxT[pair][hr, so:so + 512],
                                pc[g][sub][0:64, :], bcs[:], ALU.mult)
                        del pc[g]

                    for ti in range(NSC):
                        scores_slot(groups[0], ti)
                        yield ("pro", ti)
                    for gi, g in enumerate(groups):
                        nxt = groups[gi + 1] if gi + 1 < len(groups) else None
                        for ti in range(NSC):
                            ctx_slot(g, ti)
                            if nxt is not None:
                                scores_slot(nxt, ti)
                            yield ("slot", gi, ti)
                        norm(g)
                        yield ("norm", gi)

                def emit_tail(it):
                    """Wo + residual + LN1 -> h16 fp16, per s-chunk."""
                    st = state[it]
                    ctxT = st["ctxT"]
                    for sc in range(NSC):
                        xpt = ap.tile([128, 512], F32, tag="xpt", bufs=2)
                        nc.sync.dma_start(xpt[:], d_xp[it, sc])
                        pw = ps.tile([128, 512], F32, tag="pc", bufs=2)
                        for c in range(4):
                            nc.tensor.matmul(
                                pw[:], ctxT[c][:, sc * 128:(sc + 1) * 128],
                                t_wo[c][:], start=(c == 0), stop=(c == 3))
                        r = ap.tile([128, 512], F32, tag="res", bufs=3,
                                    name="r")
                        nc.vector.tensor_tensor(r[:], pw[:], xpt[:], ALU.add)
                        st6 = ap.tile([128, 6], F32, tag="st6", bufs=2)
                        mv = ap.tile([128, 2], F32, tag="mv", bufs=2)
                        nc.vector.bn_stats(st6[:], r[:])
                        nc.vector.bn_aggr(mv[:], st6[:])
                        inv = ap.tile([128, 1], F32, tag="st4", bufs=2)
                        nc.scalar.activation(inv[:], mv[:, 1:2], AF.Sqrt,
                                             bias=t_eps[:])
                        nc.vector.reciprocal(inv[:], inv[:])
                        if affine1:
                            ht_ = ap.tile([128, 512], F32, tag="hst", bufs=2,
                                          name="h_")
                            nc.vector.tensor_scalar(
                                ht_[:], r[:], mv[:, 0:1], inv[:],
                                ALU.subtract, ALU.mult)
                            nc.vector.tensor_tensor(ht_[:], ht_[:],
                                                    t_gb[G1][:], ALU.mult)
                            nc.vector.tensor_tensor(ht_[:], ht_[:],
                                                    t_gb[B1b][:], ALU.add)
                            nc.vector.tensor_copy(h16[it][sc][:], ht_[:])
                        else:
                            nc.vector.tensor_scalar(
                                h16[it][sc][:], r[:], mv[:, 0:1], inv[:],
                                ALU.subtract, ALU.mult)

                emit_v(0)
                for pair in range(4):
                    emit_qk(0, pair)
                emit_x(1)
                for tok in heads_gen(0):
                    if tok[0] == "norm" and tok[1] % 2 == 1:
                        emit_qk(1, tok[1] // 2)
                emit_v(1)
                emit_tail(0)
                for _ in heads_gen(1):
                    pass
                emit_tail(1)

            # =============== conv1 phase ===============
            with tc.tile_pool(name="conv1p", bufs=1) as vp, \
                 tc.tile_pool(name="psC", bufs=1, space="PSUM") as ps2:

                # ---------- B1: forward transform of h16 ----------
                V1 = [[vp.tile([128, NSC * U1B], FP16, tag=f"v1_{it}_{dc}",
                               name=f"v1_{it}_{dc}")
                       for dc in range(NDC)] for it in range(NIT)]
                for it in range(NIT):
                    for dc in range(NDC):
                        dsl = slice(dc * 128, (dc + 1) * 128)
                        for sc in range(NSC):
                            pv = ps2.tile([128, U1B + 2 * NT1], F32,
                                          tag="pb2", bufs=2)
                            nmm = 1 + (sc > 0) + (sc < NSC - 1)
                            nc.tensor.matmul(pv[:, 0:U1B],
                                             h16[it][sc][:, dsl],
                                             t_bm1[:, 0:U1B],
                                             start=True, stop=(nmm == 1))
                            i = 1
                            if sc > 0:
                                i += 1
                                nc.tensor.matmul(
                                    pv[:, U1B:U1B + NT1],
                                    h16[it][sc - 1][:, dsl],
                                    t_bm1[:, U1B:U1B + NT1],
                                    start=False, stop=(i == nmm))
                            if sc < NSC - 1:
                                i += 1
                                nc.tensor.matmul(
                                    pv[:, U1B + NT1:U1B + 2 * NT1],
                                    h16[it][sc + 1][:, dsl],
                                    t_bm1[:, U1B + NT1:U1B + 2 * NT1],
                                    start=False, stop=(i == nmm))
                            vdst = V1[it][dc][:].rearrange(
                                "p (t s) -> p t s", t=NT1, s=128)
                            pmain = pv[:, 0:U1B].rearrange(
                                "p (t tl) -> p t tl", t=NT1, tl=16)
                            nc.vector.tensor_copy(
                                vdst[:, :, sc * 16:(sc + 1) * 16], pmain)
                            if sc > 0:
                                nc.vector.tensor_tensor(
                                    vdst[:, :, sc * 16:sc * 16 + 1],
                                    vdst[:, :, sc * 16:sc * 16 + 1],
                                    pv[:, U1B:U1B + NT1].rearrange(
                                        "p (t o) -> p t o", t=NT1, o=1),
                                    ALU.add)
                            if sc < NSC - 1:
                                nc.vector.tensor_tensor(
                                    vdst[:, :, sc * 16 + 15:sc * 16 + 16],
                                    vdst[:, :, sc * 16 + 15:sc * 16 + 16],
                                    pv[:, U1B + NT1:U1B + 2 * NT1].rearrange(
                                        "p (t o) -> p t o", t=NT1, o=1),
                                    ALU.add)

                # ---------- pw1 + Karatsuba-combine + regroup + A1 ------
                # slots grouped per bin; P1,P2,P3 combined to (Mr,Mi) on
                # DVE before the DRAM trip -> M1 has 16 rows, A1 = t_a2.
                # DRAM layout: [cog, it, tau, m*512+c] so stores pair rows
                # and loads regroup with one 3D-AP DMA per (it, cog, b).
                d_m1t = dp.tile([4, NIT, 128, NT2 * 512], FP16, tag="m1d",
                                name="m1d")
                # (slot list, m rows) per bin-group
                bingrp = [([0], [0]), ([1], [1])]
                for f in range(1, 8):
                    b1 = 2 + 3 * (f - 1)
                    b2 = 2 + 2 * (f - 1)
                    bingrp.append(([b1, b1 + 1, b1 + 2], [b2, b2 + 1]))

                def _pw1_mms(pp, it, t, ut, with_bias, cog):
                    for dc in range(NDC):
                        lhs = V1[it][dc][:, t * 128:(t + 1) * 128]
                        nc.tensor.matmul(
                            pp[:], lhs, ut[:, dc * 512:(dc + 1) * 512],
                            start=(dc == 0),
                            stop=(dc == NDC - 1 and not with_bias))
                    if with_bias:
                        nc.tensor.matmul(
                            pp[:], t_ones[0:1, :],
                            t_bc1r[0:1, cog * 512:(cog + 1) * 512],
                            start=False, stop=True)

                for cog in range(4):
                    for slots, mrows in bingrp:
                        uts = []
                        for t in slots:
                            ut = vp.tile([128, NDC * 512], FP16, tag="u1s",
                                         bufs=12, name="u1t")
                            nc.sync.dma_start(ut[:], d_u1[cog, t])
                            uts.append(ut)
                        for it in range(NIT):
                            pps = []
                            for k, t in enumerate(slots):
                                pp = ps2.tile([128, 512], F32, tag="ppw",
                                              bufs=4)
                                _pw1_mms(pp, it, t, uts[k],
                                         t == 0, cog)
                                pps.append(pp)
                            if len(slots) == 1:
                                mg = vp.tile([128, 512], FP16, tag="m1g",
                                             bufs=4, name="mg")
                                nc.vector.tensor_copy(mg[:], pps[0][:])
                                mo = mrows[0] * 512
                                nc.scalar.dma_start(
                                    d_m1t[cog, it][:, mo:mo + 512], mg[:])
                            else:
                                p1, p2, p3 = pps
                                p2s = vp.tile([128, 512], F32, tag="p2s",
                                              bufs=2, name="p2s")
                                nc.vector.tensor_copy(p2s[:], p2[:])
                                mp = vp.tile([128, 1024], FP16, tag="m1p",
                                             bufs=4, name="mp")
                                nc.vector.tensor_tensor(mp[:, 0:512], p1[:],
                                                        p2s[:], ALU.subtract)
                                tmp = vp.tile([128, 512], F32, tag="mtmp",
                                              bufs=2, name="mtmp")
                                nc.vector.tensor_tensor(tmp[:], p3[:],
                                                        p2s[:], ALU.subtract)
                                nc.vector.tensor_tensor(mp[:, 512:1024],
                                                        tmp[:], p1[:],
                                                        ALU.subtract)
                                mo = mrows[0] * 512
                                nc.scalar.dma_start(
                                    d_m1t[cog, it][:, mo:mo + 1024], mp[:])
                    for it in range(NIT):
                        for b in range(16):
                            mh0 = vp.tile([128, 512], FP16, tag="mh0",
                                          bufs=4)
                            nc.scalar.dma_start(
                                mh0[:],
                                d_m1t[cog, it][b * 8:(b + 1) * 8, :]
                                .rearrange("u (m c) -> m u c",
                                           m=NT2, c=512))
                            pa = ps2.tile([64, 512], F32, tag="pa", bufs=2)
                            nc.tensor.matmul(pa[:], t_a2[:, :], mh0[:, :],
                                             start=True, stop=True)
                            c1s = vp.tile([64, 512], FP16, tag="c1s", bufs=4)
                            nc.scalar.activation(c1s[:], pa[:], AF.Relu)
                            ro = (b % 2) * 64
                            nc.gpsimd.dma_start(
                                c1_dram[it][b // 2][ro:ro + 64,
                                                    cog * 512:
                                                    (cog + 1) * 512],
                                c1s[:])

            # =============== conv2 phase ===============
            with tc.tile_pool(name="conv2p", bufs=1) as vp, \
                 tc.tile_pool(name="psD", bufs=1, space="PSUM") as ps2:

                d_m2t = dp.tile([NIT, 128, NT2 * 512], FP16, tag="m2d",
                                name="m2d")

                # ---------- B2: forward transform of c1 ----------
                V2 = [[vp.tile([128, NSC * U2B], FP16, tag=f"v2_{it}_{ch}",
                               name=f"v2_{it}_{ch}")
                       for ch in range(NCD)] for it in range(NIT)]
                for it in range(NIT):
                    c1w = {}

                    def fetch(sc, it=it, c1w=c1w):
                        w = vp.tile([128, CD], FP16, tag="c1w", bufs=4,
                                    name="c1w")
                        nc.sync.dma_start(w[:], c1_dram[it][sc][:])
                        c1w[sc] = w

                    def proc(sc, it=it, c1w=c1w):
                        for ch in range(NCD):
                            csl = slice(ch * 128, (ch + 1) * 128)
                            pv = ps2.tile([128, U2B + 2 * NT2], F32,
                                          tag="pb2", bufs=2)
                            nmm = 1 + (sc > 0) + (sc < NSC - 1)
                            nc.tensor.matmul(pv[:, 0:U2B], c1w[sc][:, csl],
                                             t_bm2[:, 0:U2B],
                                             start=True, stop=(nmm == 1))
                            i = 1
                            if sc > 0:
                                i += 1
                                nc.tensor.matmul(
                                    pv[:, U2B:U2B + NT2],
                                    c1w[sc - 1][:, csl],
                                    t_bm2[:, U2B:U2B + NT2],
                                    start=False, stop=(i == nmm))
                            if sc < NSC - 1:
                                i += 1
                                nc.tensor.matmul(
                                    pv[:, U2B + NT2:U2B + 2 * NT2],
                                    c1w[sc + 1][:, csl],
                                    t_bm2[:, U2B + NT2:U2B + 2 * NT2],
                                    start=False, stop=(i == nmm))
                            vdst = V2[it][ch][:].rearrange(
                                "p (t s) -> p t s", t=NT2, s=128)
                            pmain = pv[:, 0:U2B].rearrange(
                                "p (t tl) -> p t tl", t=NT2, tl=16)
                            nc.vector.tensor_copy(
                                vdst[:, :, sc * 16:(sc + 1) * 16], pmain)
                            if sc > 0:
                                nc.vector.tensor_tensor(
                                    vdst[:, :, sc * 16:sc * 16 + 1],
                                    vdst[:, :, sc * 16:sc * 16 + 1],
                                    pv[:, U2B:U2B + NT2].rearrange(
                                        "p (t o) -> p t o", t=NT2, o=1),
                                    ALU.add)
                            if sc < NSC - 1:
                                nc.vector.tensor_tensor(
                                    vdst[:, :, sc * 16 + 15:sc * 16 + 16],
                                    vdst[:, :, sc * 16 + 15:sc * 16 + 16],
                                    pv[:, U2B + NT2:U2B + 2 * NT2].rearrange(
                                        "p (t o) -> p t o", t=NT2, o=1),
                                    ALU.add)

                    fetch(0)
                    fetch(1)
                    proc(0)
                    for sc in range(1, NSC - 1):
                        fetch(sc + 1)
                        proc(sc)
                    proc(NSC - 1)

                # ---------- pw2 ----------
                mp2 = {}
                for m in range(NT2):
                    slots = [i for i, sl in enumerate(SL2) if sl[4] == m]
                    pps = [ps2.tile([128, 512], F32, tag="ppw", bufs=4,
                                    name=f"pp2{it}")
                           for it in range(NIT)]
                    for k, si in enumerate(slots):
                        vrow = SL2[si][3]
                        for g in range(4):
                            ut = vp.tile([128, 4 * 512], FP16, tag="u2s",
                                         bufs=6, name="u2t")
                            nc.sync.dma_start(ut[:], d_u2[si, g])
                            for c4 in range(4):
                                ch = g * 4 + c4
                                for it in range(NIT):
                                    lhs = V2[it][ch][:,
                                        vrow * 128:(vrow + 1) * 128]
                                    islast = (k == len(slots) - 1 and
                                              ch == NCD - 1 and m != 0)
                                    nc.tensor.matmul(
                                        pps[it][:], lhs,
                                        ut[:, c4 * 512:(c4 + 1) * 512],
                                        start=(k == 0 and ch == 0),
                                        stop=islast)
                    if m == 0:
                        for it in range(NIT):
                            nc.tensor.matmul(pps[it][:], t_ones[0:1, :],
                                             t_bc2r[0:1, :],
                                             start=False, stop=True)
                    for it in range(NIT):
                        if m % 2 == 0:
                            mp2[it] = vp.tile([128, 1024], FP16, tag="m2s",
                                              bufs=4, name="mg2")
                            nc.vector.tensor_copy(mp2[it][:, 0:512],
                                                  pps[it][:])
                        else:
                            nc.vector.tensor_copy(mp2[it][:, 512:1024],
                                                  pps[it][:])
                            mo = (m - 1) * 512
                            nc.scalar.dma_start(
                                d_m2t[it][:, mo:mo + 1024], mp2[it][:])

            # ---------- regroup2 + A2 + ReLU + residual + LN2 -------
            with tc.tile_pool(name="conv2b", bufs=1) as vp, \
                 tc.tile_pool(name="psE", bufs=1, space="PSUM") as ps2:
                for it in range(NIT):
                    eng = nc.sync if it == 0 else nc.scalar
                    for sc in range(NSC):
                        pa2 = ps2.tile([128, 512], F32, tag="pa", bufs=2)
                        for half in range(2):
                            b = 2 * sc + half
                            mh2 = vp.tile([128, 512], FP16, tag="mh2",
                                          bufs=4)
                            eng.dma_start(
                                mh2[:],
                                d_m2t[it][b * 8:(b + 1) * 8, :]
                                .rearrange("u (m c) -> m u c",
                                           m=NT2, c=512))
                            ro = half * 64
                            nc.tensor.matmul(pa2[ro:ro + 64, :], t_a2[:, :],
                                             mh2[:, :], start=True,
                                             stop=True)
                        t1 = vp.tile([128, 512], F32, tag="t1", bufs=3)
                        nc.scalar.activation(t1[:], pa2[:], AF.Relu)
                        r = vp.tile([128, 512], F32, tag="r2", bufs=4)
                        nc.vector.tensor_tensor(r[:], t1[:], h16[it][sc][:],
                                                ALU.add)
                        st6 = vp.tile([128, 6], F32, tag="st62", bufs=2)
                        mv = vp.tile([128, 2], F32, tag="mv2", bufs=2)
                        nc.vector.bn_stats(st6[:], r[:])
                        nc.vector.bn_aggr(mv[:], st6[:])
                        inv = vp.tile([128, 1], F32, tag="inv2", bufs=2)
                        nc.scalar.activation(inv[:], mv[:, 1:2], AF.Sqrt,
                                             bias=t_eps[:])
                        nc.vector.reciprocal(inv[:], inv[:])
                        yt = vp.tile([128, 512], F32, tag="yt2", bufs=3)
                        nc.vector.tensor_scalar(yt[:], r[:], mv[:, 0:1],
                                                inv[:], ALU.subtract,
                                                ALU.mult)
                        if affine2:
                            nc.vector.tensor_tensor(yt[:], yt[:],
                                                    t_gb[G2][:], ALU.mult)
                            nc.vector.tensor_tensor(yt[:], yt[:],
                                                    t_gb[B2b][:], ALU.add)
                        nc.sync.dma_start(d_y[it, sc], yt[:])

    nc.compile()
    _BUILT[key] = nc
    return nc


def _prep_host(inputs):
    x = np.asarray(inputs["x"], np.float32)
    Wq = np.asarray(inputs["Wq"], np.float32)
    bq = np.asarray(inputs["bq"], np.float32)
    Wk = np.asarray(inputs["Wk"], np.float32)
    bk = np.asarray(inputs["bk"], np.float32)
    Wv = np.asarray(inputs["Wv"], np.float32)
    bv = np.asarray(inputs["bv"], np.float32)
    Wo = np.asarray(inputs["Wo"], np.float32)
    bo = np.asarray(inputs["bo"], np.float32)
    g1 = np.asarray(inputs["g1"], np.float32)
    b1 = np.asarray(inputs["b1"], np.float32)
    g2 = np.asarray(inputs["g2"], np.float32)
    b2 = np.asarray(inputs["b2"], np.float32)
    Wc1 = np.asarray(inputs["Wc1"], np.float64)
    bc1 = np.asarray(inputs["bc1"], np.float32)
    Wc2 = np.asarray(inputs["Wc2"], np.float64)
    bc2 = np.asarray(inputs["bc2"], np.float32)

    xT = np.ascontiguousarray(x.transpose(0, 2, 1).reshape(B, NDC, 128, S))
    xp = np.ascontiguousarray((x + bo[None, None, :]).reshape(B, NSC, 128, D))

    wqk = np.zeros((2, 4, 128, 512), np.float32)
    for proj, W in ((0, Wq), (1, Wk)):
        for pair in range(4):
            blk = np.concatenate([W[2 * pair], W[2 * pair + 1]], axis=1)
            wqk[proj, pair] = blk.reshape(NDC, 128, 128).transpose(1, 0, 2) \
                                 .reshape(128, 512)
    bqk = np.zeros((128, 8), np.float32)
    for proj, b in ((0, bq), (1, bk)):
        for pair in range(4):
            bqk[:, proj * 4 + pair] = np.concatenate(
                [b[2 * pair], b[2 * pair + 1]])

    wv = np.zeros((NDC, 128, 520), np.float32)
    bvrow = np.zeros((128, 520), np.float32)
    for h in range(H):
        wv[:, :, h * 65:h * 65 + 64] = Wv[h].reshape(NDC, 128, 64)
        bvrow[:, h * 65:h * 65 + 64] = bv[h][None, :]
        bvrow[:, h * 65 + 64] = 1.0

    wo = np.ascontiguousarray(Wo.reshape(4, 128, 512))

    # ---- conv transform constants ----
    BT1, AT1, BT2, AT2, SL2 = _dft_mats()
    bm1 = _bmat(BT1).astype(np.float16)
    bm2 = _bmat(BT2).astype(np.float16)
    a0 = _amat_packed(AT1, list(range(12))).astype(np.float16)
    a1 = _amat_packed(AT1, list(range(12, NT1))).astype(np.float16)
    a2 = _amat_packed(AT2, list(range(NT2))).astype(np.float16)
    ones = np.ones((1, 128), np.float16)
    bc1r = bc1.reshape(1, CD).astype(np.float16)
    bc2r = bc2.reshape(1, D).astype(np.float16)

    U1 = _u_slots1(Wc1)                              # [23, 512, 2048]
    u1 = np.ascontiguousarray(
        U1.reshape(NT1, NDC, 128, 4, 512).transpose(3, 0, 2, 1, 4)
          .reshape(4, NT1, 128, NDC * 512)).astype(np.float16)
    U2 = _u_slots2(Wc2, SL2)                         # [30, 2048, 512]
    u2 = np.ascontiguousarray(
        U2.reshape(NSL2, 4, 4, 128, 512).transpose(0, 1, 3, 2, 4)
          .reshape(NSL2, 4, 128, 4 * 512)).astype(np.float16)

    gb = np.stack([np.tile(v[None, :], (128, 1))
                   for v in (g1, b1, g2, b2)]).astype(np.float32)
    cones = np.ones((128, 128), np.float32)

    shared = dict(wqk=wqk, bqk=bqk, wv=wv, bvrow=bvrow, wo=wo, gb=gb,
                  cones=cones, bm1=bm1, bm2=bm2, a0=a0, a1=a1, a2=a2,
                  ones=ones, bc1r=bc1r, bc2r=bc2r, u1=u1, u2=u2)
    in_maps = []
    for c in range(NCORES):
        m = dict(shared)
        m["xT"] = np.ascontiguousarray(xT[c * NIT:(c + 1) * NIT])
        m["xp"] = np.ascontiguousarray(xp[c * NIT:(c + 1) * NIT])
        in_maps.append(m)
    return in_maps


def run(inputs, trace=False, **trace_kwargs):
    affine1 = not (np.all(np.asarray(inputs["g1"]) == 1.0)
                   and np.all(np.asarray(inputs["b1"]) == 0.0))
    affine2 = not (np.all(np.asarray(inputs["g2"]) == 1.0)
                   and np.all(np.asarray(inputs["b2"]) == 0.0))
    nc = _build(affine1, affine2)
    from concourse.bass_utils import run_bass_kernel_spmd
    in_maps = _prep_host(inputs)
    res = run_bass_kernel_spmd(nc, in_maps, core_ids=list(range(NCORES)),
                               trace=trace, **trace_kwargs)
    y = np.concatenate([res.results[c]["y"].reshape(NIT, S, D)
                        for c in range(NCORES)], axis=0)
    return y, res


def kernel(**inputs):
    y, _ = run(inputs, trace=False)
    return y


# revision 28
# speedup vs baseline: 1.1577x; 1.1577x over previous
"""Trainium2 Bass kernel for the FFT-block (attention + conv FFN) problem.

Sharding: data-parallel over batch. B=16 items across 8 cores -> 2 items/core.

v3: the conv FFN (85% of FLOPs) is computed via length-16 overlap-save
cyclic convolution in the real-DFT domain (fp16 transform-domain matmuls,
fp32 PSUM accumulation):
  - conv1 uses Karatsuba 3-mult complex products -> 23 mult-slots per 8
    outputs (vs 72 dense matmul slots); the Karatsuba recombination is
    folded into the inverse-transform (A) matrix.
  - conv2 uses plain 4-mult products -> 30 slots, which keeps the V2
    tensor small enough to hold both items in SBUF so U2 streams once.
  - B-stage (forward transform) / A-stage (inverse transform) are matmuls
    against constant matrices; conv biases fold in as K=1 matmuls on the
    DC slot.
  - M tensors are regrouped tau-major -> t-major with SBUF->SBUF DMAs so
    the A-stage contraction (over transform slots) runs on the PE.
DFT transforms are orthogonal, so fp16 transform-domain numerics land at
~6e-4 max-rel (vs 2.1e-3 for the dense bf16 v2 kernel).
"""
import sys, types
import numpy as np

B, S, D = 16, 1024, 512
H, DK = 8, 64
CD, KS = 2048, 9
EPS = 1e-5
NCORES = 8
NIT = B // NCORES
NDC = D // 128             # 4 d-chunks
NSC = S // 128             # 8 s-chunks
NCOL = S // 512            # 2 s-cols
NCD = CD // 128            # 16 cd-chunks

N16, M8 = 16, 8            # DFT length / outputs per tile
NT1 = 23                   # conv1 slots (Karatsuba)
NT2 = 16                   # conv2 V/M rows (plain)
NSL2 = 30                  # conv2 mult slots
U1B = 16 * NT1             # 368  B1 main cols
U2B = 16 * NT2             # 256  B2 main cols


def _install_ntff_hook():
    try:
        from antenv.axon_hooks import get_axon_ntff_profile_hook  # noqa
        return
    except ImportError:
        pass
    try:
        from trn_agent_boot.trn_boot import _ntff_profile_via_ctypes
        mod = types.ModuleType('antenv.axon_hooks')
        hook = _ntff_profile_via_ctypes('/opt/axon/libaxon_pjrt.so')
        mod.get_axon_ntff_profile_hook = lambda: hook
        sys.modules['antenv.axon_hooks'] = mod
    except Exception:
        pass


# ---------------- DFT-domain transform matrices (host, float64) ---------
def _dft_mats():
    """Returns BT1 [23,16], AT1 [8,23], BT2 [16,16], AT2 [8,16], SL2."""
    j = np.arange(N16)
    BT1 = np.zeros((NT1, N16))
    BT2 = np.zeros((NT2, N16))
    cr = lambda f: np.cos(2 * np.pi * j * f / N16)
    si = lambda f: -np.sin(2 * np.pi * j * f / N16)  # Im of e^{-2pi i jf/16}
    BT1[0] = cr(0); BT1[1] = cr(8)
    BT2[0] = cr(0); BT2[1] = cr(8)
    for f in range(1, 8):
        b1 = 2 + 3 * (f - 1)
        BT1[b1 + 0] = cr(f)
        BT1[b1 + 1] = si(f)
        BT1[b1 + 2] = cr(f) + si(f)
        b2 = 2 + 2 * (f - 1)
        BT2[b2 + 0] = cr(f)
        BT2[b2 + 1] = si(f)
    AT1 = np.zeros((M8, NT1))
    AT2 = np.zeros((M8, NT2))
    for ss in range(M8):
        AT1[ss, 0] = 1.0; AT1[ss, 1] = (-1.0) ** ss
        AT2[ss, 0] = 1.0; AT2[ss, 1] = (-1.0) ** ss
        for f in range(1, 8):
            th = 2 * np.pi * ss * f / N16
            co, sn = np.cos(th), np.sin(th)
            b1 = 2 + 3 * (f - 1)
            AT1[ss, b1 + 0] = 2 * (co + sn)
            AT1[ss, b1 + 1] = 2 * (-co + sn)
            AT1[ss, b1 + 2] = -2 * sn
            b2 = 2 + 2 * (f - 1)
            AT2[ss, b2 + 0] = 2 * co
            AT2[ss, b2 + 1] = -2 * sn
    # conv2 slot list: (bin f, part, sign, vrow, mrow), m-major order
    SL2 = [(0, 'r', 1.0, 0, 0), (8, 'r', 1.0, 1, 1)]
    for f in range(1, 8):
        b2 = 2 + 2 * (f - 1)
        SL2 += [(f, 'r', 1.0, b2 + 0, b2 + 0),     # Mr += Ur*Vr
                (f, 'i', -1.0, b2 + 1, b2 + 0),    # Mr -= Ui*Vi
                (f, 'r', 1.0, b2 + 1, b2 + 1),     # Mi += Ur*Vi
                (f, 'i', 1.0, b2 + 0, b2 + 1)]     # Mi += Ui*Vr
    return BT1, AT1, BT2, AT2, SL2


def _u_slots1(W):
    """W [CD, D, 9] -> U1 [23, D, CD] (1/16 folded)."""
    gp = np.zeros(W.shape[:2] + (N16,))
    gp[:, :, :KS] = W
    Gh = np.conj(np.fft.fft(gp, axis=2)) / N16      # [co, ci, 16]
    Ut = Gh.transpose(2, 1, 0)                      # [16, ci, co]
    U = np.zeros((NT1,) + Ut.shape[1:])
    U[0] = Ut[0].real; U[1] = Ut[8].real
    for f in range(1, 8):
        b1 = 2 + 3 * (f - 1)
        U[b1 + 0] = Ut[f].real
        U[b1 + 1] = Ut[f].imag
        U[b1 + 2] = Ut[f].real + Ut[f].imag
    return U


def _u_slots2(W, SL2):
    """W [D, CD, 9] -> U2 [30, CD, D] with signs and 1/16 folded."""
    gp = np.zeros(W.shape[:2] + (N16,))
    gp[:, :, :KS] = W
    Gh = np.conj(np.fft.fft(gp, axis=2)) / N16      # [co, ci, 16]
    Ut = Gh.transpose(2, 1, 0)                      # [16, ci, co]
    U = np.zeros((NSL2,) + Ut.shape[1:])
    for i, (f, part, sign, vrow, mrow) in enumerate(SL2):
        U[i] = sign * (Ut[f].real if part == 'r' else Ut[f].imag)
    return U


def _bmat(BT):
    """BT [nt, 16] -> Bmat [128, nt*16 + 2*nt] (main|prev|next cols)."""
    nt = BT.shape[0]
    nb = nt * 16
    Bm = np.zeros((128, nb + 2 * nt))
    for tl in range(16):
        for t in range(nt):
            for jj in range(N16):
                ss = 8 * tl + jj - 4
                if 0 <= ss < 128:
                    Bm[ss, t * 16 + tl] = BT[t, jj]
    for t in range(nt):
        for jj in range(4):                 # prev: rows 124..127 <- j 0..3
            Bm[124 + jj, nb + t] = BT[t, jj]
        for ss in range(4):                 # next: rows 0..3 <- j 12..15
            Bm[ss, nb + nt + t] = BT[t, 12 + ss]
    return Bm


def _amat_packed(AT, tlist):
    """A[i*8+tau, tau2*8+s] = AT[s, tlist[i]] * (tau==tau2) -> [n*8, 64]."""
    n = len(tlist)
    A = np.zeros((n * 8, 64))
    for i, t in enumerate(tlist):
        for tau in range(8):
            for ss in range(M8):
                A[i * 8 + tau, tau * M8 + ss] = AT[ss, t]
    return A


_BUILT = {}


def _build(affine1=False, affine2=False):
    """affine1/affine2: apply LN1/LN2 gamma,beta (skipped when g==1, b==0)."""
    global _BUILT
    key = (affine1, affine2)
    if key in _BUILT:
        return _BUILT[key]
    _install_ntff_hook()
    import concourse.bacc as bacc
    import concourse.mybir as mybir
    from concourse import tile
    from concourse.masks import make_identity
    from contextlib import ExitStack

    F32 = mybir.dt.float32
    F32R = mybir.dt.float32r
    BF16 = mybir.dt.bfloat16
    FP16 = mybir.dt.float16
    AF = mybir.ActivationFunctionType
    ALU = mybir.AluOpType

    _, _, _, _, SL2 = _dft_mats()

    nc = bacc.Bacc("TRN2", target_bir_lowering=False, debug=False,
                   num_devices=NCORES)

    # ---- DRAM I/O (per core) ----
    d_xT = nc.dram_tensor("xT", [NIT, NDC, 128, S], F32, kind="ExternalInput")
    d_xp = nc.dram_tensor("xp", [NIT, NSC, 128, D], F32, kind="ExternalInput")
    d_wqk = nc.dram_tensor("wqk", [2, 4, 128, 512], F32, kind="ExternalInput")
    d_bqk = nc.dram_tensor("bqk", [128, 8], F32, kind="ExternalInput")
    d_wv = nc.dram_tensor("wv", [NDC, 128, 520], F32, kind="ExternalInput")
    d_bvrow = nc.dram_tensor("bvrow", [128, 520], F32, kind="ExternalInput")
    d_wo = nc.dram_tensor("wo", [4, 128, 512], F32, kind="ExternalInput")
    d_gb = nc.dram_tensor("gb", [4, 128, 512], F32, kind="ExternalInput")
    d_cones = nc.dram_tensor("cones", [128, 128], F32, kind="ExternalInput")
    d_bm1 = nc.dram_tensor("bm1", [128, U1B + 2 * NT1], FP16,
                           kind="ExternalInput")
    d_bm2 = nc.dram_tensor("bm2", [128, U2B + 2 * NT2], FP16,
                           kind="ExternalInput")
    d_a0 = nc.dram_tensor("a0", [96, 64], FP16, kind="ExternalInput")
    d_a1 = nc.dram_tensor("a1", [88, 64], FP16, kind="ExternalInput")
    d_a2 = nc.dram_tensor("a2", [128, 64], FP16, kind="ExternalInput")
    d_ones = nc.dram_tensor("ones", [1, 128], FP16, kind="ExternalInput")
    d_bc1r = nc.dram_tensor("bc1r", [1, CD], FP16, kind="ExternalInput")
    d_bc2r = nc.dram_tensor("bc2r", [1, D], FP16, kind="ExternalInput")
    d_u1 = nc.dram_tensor("u1", [4, NT1, 128, NDC * 512], FP16,
                          kind="ExternalInput")
    d_u2 = nc.dram_tensor("u2", [NSL2, 4, 128, 4 * 512], FP16,
                          kind="ExternalInput")
    d_y = nc.dram_tensor("y", [NIT, NSC, 128, D], F32, kind="ExternalOutput")

    G1, B1b, G2, B2b = range(4)

    with tile.TileContext(nc) as tc:
        est = ExitStack()
        with est:
            cp = est.enter_context(tc.tile_pool(name="const", bufs=1))
            kp = est.enter_context(tc.tile_pool(name="keep", bufs=1))
            dp = est.enter_context(tc.tile_pool(name="dramp", bufs=1,
                                                space="DRAM"))

            # c1 spill (DRAM scratch), per (item, s-chunk)
            c1_dram = [[dp.tile([128, CD], FP16, tag=f"c1d{it}_{sc}",
                                name=f"c1d{it}_{sc}")
                        for sc in range(NSC)] for it in range(NIT)]

            # ---- persistent constants ----
            t_eps = cp.tile([128, 1], F32, tag="eps")
            nc.vector.memset(t_eps[:], EPS)
            t_gb = []
            for i in range(4):
                t = cp.tile([128, 512], F32, tag=f"gb{i}", name=f"gb{i}")
                if affine1 or affine2:
                    nc.sync.dma_start(t[:], d_gb[i])
                t_gb.append(t)
            t_bm1 = cp.tile([128, U1B + 2 * NT1], FP16, tag="bm1")
            nc.sync.dma_start(t_bm1[:], d_bm1[:])
            t_bm2 = cp.tile([128, U2B + 2 * NT2], FP16, tag="bm2")
            nc.sync.dma_start(t_bm2[:], d_bm2[:])
            t_a0 = cp.tile([96, 64], FP16, tag="a0")
            nc.sync.dma_start(t_a0[:], d_a0[:])
            t_a1 = cp.tile([88, 64], FP16, tag="a1")
            nc.sync.dma_start(t_a1[:], d_a1[:])
            t_a2 = cp.tile([128, 64], FP16, tag="a2")
            nc.sync.dma_start(t_a2[:], d_a2[:])
            t_ones = cp.tile([1, 128], FP16, tag="onesr")
            nc.sync.dma_start(t_ones[:], d_ones[:])
            t_bc1r = cp.tile([1, CD], FP16, tag="bc1r")
            nc.sync.dma_start(t_bc1r[:], d_bc1r[:])
            t_bc2r = cp.tile([1, D], FP16, tag="bc2r")
            nc.sync.dma_start(t_bc2r[:], d_bc2r[:])

            # LN1 output in fp16, persistent across phases
            h16 = [[kp.tile([128, D], FP16, tag=f"h16_{it}_{sc}",
                            name=f"h16_{it}_{sc}")
                    for sc in range(NSC)] for it in range(NIT)]

            state = [dict() for _ in range(NIT)]

            # =============== attention phase ===============
            with tc.tile_pool(name="attn", bufs=1) as ap, \
                 tc.tile_pool(name="psA", bufs=1, space="PSUM") as ps:

                def emit_x(it):
                    st = state[it]
                    xt = []
                    for dc in range(NDC):
                        t = ap.tile([128, S], F32R, tag=f"xt{dc}",
                                    name=f"xt{dc}")
                        nc.sync.dma_start(t[:], d_xT[it, dc].bitcast(F32R))
                        xt.append(t)
                    st["xt"] = xt
                    st["qkt"] = {}

                emit_x(0)

                t_wv = []
                for dc in range(NDC):
                    t = ap.tile([128, 520], F32R, tag=f"wv{dc}",
                                name=f"wv{dc}")
                    nc.sync.dma_start(t[:], d_wv[dc].bitcast(F32R))
                    t_wv.append(t)
                t_bvfull = ap.tile([128, 520], F32, tag="bvfull")
                nc.sync.dma_start(t_bvfull[:], d_bvrow[:])
                t_bqk = ap.tile([128, 8], F32, tag="bqk")
                nc.sync.dma_start(t_bqk[:], d_bqk[:])
                t_cones = ap.tile([128, 128], F32R, tag="cones")
                nc.sync.dma_start(t_cones[:], d_cones[:].bitcast(F32R))
                t_wo = []
                for c in range(4):
                    t = ap.tile([128, 512], F32R, tag=f"wo{c}", name=f"wo{c}")
                    nc.sync.dma_start(t[:], d_wo[c].bitcast(F32R))
                    t_wo.append(t)

                def emit_v(it):
                    st = state[it]
                    xt = st["xt"]
                    vst = []
                    for tc_i in range(NSC):
                        vt = ap.tile([128, 520], BF16, tag=f"vst{tc_i}",
                                     name=f"vst{tc_i}")
                        for half in range(2):
                            colo = half * 260
                            pv = ps.tile([128, 260], F32, tag="pp", bufs=2)
                            for dc in range(NDC):
                                nc.tensor.matmul(
                                    pv[:],
                                    xt[dc][:, tc_i * 128:(tc_i + 1) * 128],
                                    t_wv[dc][:, colo:colo + 260],
                                    start=(dc == 0), stop=(dc == NDC - 1))
                            nc.vector.tensor_tensor(
                                vt[:, colo:colo + 260], pv[:],
                                t_bvfull[:, colo:colo + 260], ALU.add)
                        vst.append(vt)
                    st["vst"] = vst

                def emit_qk(it, pair):
                    st = state[it]
                    xt = st["xt"]
                    for proj in range(2):
                        wt = ap.tile([128, 512], F32R, tag=f"wqk{proj}",
                                     bufs=2, name="wt")
                        nc.sync.dma_start(wt[:],
                                          d_wqk[proj, pair].bitcast(F32R))
                        qt = ap.tile([128, S], BF16, tag=f"qk{proj}{pair}",
                                     name="qt")
                        for scol in range(NCOL):
                            pq = ps.tile([128, 512], F32, tag="pp", bufs=2)
                            for dc in range(NDC):
                                nc.tensor.matmul(
                                    pq[:], wt[:, dc * 128:(dc + 1) * 128],
                                    xt[dc][:, scol * 512:(scol + 1) * 512],
                                    start=(dc == 0), stop=(dc == NDC - 1))
                            nc.vector.tensor_scalar_add(
                                qt[:, scol * 512:(scol + 1) * 512], pq[:],
                                t_bqk[:, proj * 4 + pair:proj * 4 + pair + 1])
                        st["qkt"][(proj, pair)] = qt

                def heads_gen(it):
                    st = state[it]
                    st["ctxT"] = [ap.tile([128, S], F32R, tag=f"ct{c}",
                                          name=f"ct{c}") for c in range(4)]
                    ctxT = st["ctxT"]
                    vst = st["vst"]
                    groups = [(p, s) for p in range(4) for s in range(2)]
                    pex = {}
                    pc = {}

                    def scores_slot(g, ti):
                        pair, scol = g
                        so = scol * 512
                        qT = st["qkt"][(0, pair)]
                        kT = st["qkt"][(1, pair)]
                        sc2 = ps.tile([128, 1024], F32, tag="sc2", bufs=2,
                                      name="sc2")
                        for sub in range(2):
                            hr = slice(sub * 64, sub * 64 + 64)
                            nc.tensor.matmul(
                                sc2[:, sub * 512:(sub + 1) * 512],
                                kT[hr, ti * 128:(ti + 1) * 128],
                                qT[hr, so:so + 512], start=True, stop=True)
                        pe = ap.tile([128, 1024], BF16, tag=f"pex{ti}",
                                     bufs=1, name="pe")
                        nc.scalar.activation(pe[:], sc2[:], AF.Exp,
                                             scale=0.125)
                        pex[ti] = pe

                    def ctx_slot(g, ti):
                        pair, scol = g
                        if ti == 0:
                            pc[g] = [ps.tile([65, 512], F32, tag="pc",
                                             bufs=2, name=f"pcx{sub}")
                                     for sub in range(2)]
                        for sub in range(2):
                            h = 2 * pair + sub
                            nc.tensor.matmul(
                                pc[g][sub][:], vst[ti][:, h * 65:h * 65 + 65],
                                pex[ti][:, sub * 512:(sub + 1) * 512],
                                start=(ti == 0), stop=(ti == NSC - 1))

                    def norm(g):
                        pair, scol = g
                        so = scol * 512
                        for sub in range(2):
                            hr = slice(sub * 64, sub * 64 + 64)
                            zr = ap.tile([1, 512], F32R, tag="bcs", bufs=4,
                                         name="zr")
                            nc.vector.tensor_copy(zr[0:1, :],
                                                  pc[g][sub][64:65, :])
                            pb = ps.tile([64, 512], F32, tag="pp", bufs=2,
                                         name="pb")
                            nc.tensor.matmul(pb[:], t_cones[0:1, 0:64],
                                             zr[0:1, :], start=True,
                                             stop=True)
                            bcs = ap.tile([64, 512], F32, tag="bcs2", bufs=4,
                                          name="bcs")
                            nc.vector.reciprocal_approx_fast(out=bcs[:],
                                                             in_=pb[:])
                            nc.vector.tensor_tensor(
                                ctxT[pair][hr, so:so + 512],
                                pc[g][sub][0:64, :], bcs[:], ALU.mult)
                        del pc[g]

                    for ti in range(NSC):
                        scores_slot(groups[0], ti)
                        yield ("pro", ti)
                    for gi, g in enumerate(groups):
                        nxt = groups[gi + 1] if gi + 1 < len(groups) else None
                        for ti in range(NSC):
                            ctx_slot(g, ti)
                            if nxt is not None:
                                scores_slot(nxt, ti)
                            yield ("slot", gi, ti)
                        norm(g)
                        yield ("norm", gi)

                def emit_tail(it):
                    """Wo + residual + LN1 -> h16 fp16, per s-chunk."""
                    st = state[it]
                    ctxT = st["ctxT"]
                    for sc in range(NSC):
                        xpt = ap.tile([128, 512], F32, tag="xpt", bufs=2)
                        nc.sync.dma_start(xpt[:], d_xp[it, sc])
                        pw = ps.tile([128, 512], F32, tag="pc", bufs=2)
                        for c in range(4):
                            nc.tensor.matmul(
                                pw[:], ctxT[c][:, sc * 128:(sc + 1) * 128],
                                t_wo[c][:], start=(c == 0), stop=(c == 3))
                        r = ap.tile([128, 512], F32, tag="res", bufs=3,
                                    name="r")
                        nc.vector.tensor_tensor(r[:], pw[:], xpt[:], ALU.add)
                        st6 = ap.tile([128, 6], F32, tag="st6", bufs=2)
                        mv = ap.tile([128, 2], F32, tag="mv", bufs=2)
                        nc.vector.bn_stats(st6[:], r[:])
                        nc.vector.bn_aggr(mv[:], st6[:])
                        inv = ap.tile([128, 1], F32, tag="st4", bufs=2)
                        nc.scalar.activation(inv[:], mv[:, 1:2], AF.Sqrt,
                                             bias=t_eps[:])
                        nc.vector.reciprocal(inv[:], inv[:])
                        if affine1:
                            ht_ = ap.tile([128, 512], F32, tag="hst", bufs=2,
                                          name="h_")
                            nc.vector.tensor_scalar(
                                ht_[:], r[:], mv[:, 0:1], inv[:],
                                ALU.subtract, ALU.mult)
                            nc.vector.tensor_tensor(ht_[:], ht_[:],
                                                    t_gb[G1][:], ALU.mult)
                            nc.vector.tensor_tensor(ht_[:], ht_[:],
                                                    t_gb[B1b][:], ALU.add)
                            nc.vector.tensor_copy(h16[it][sc][:], ht_[:])
                        else:
                            nc.vector.tensor_scalar(
                                h16[it][sc][:], r[:], mv[:, 0:1], inv[:],
                                ALU.subtract, ALU.mult)

                emit_v(0)
                for pair in range(4):
                    emit_qk(0, pair)
                emit_x(1)
                for tok in heads_gen(0):
                    if tok[0] == "norm" and tok[1] % 2 == 1:
                        emit_qk(1, tok[1] // 2)
                emit_v(1)
                emit_tail(0)
                for _ in heads_gen(1):
                    pass
                emit_tail(1)

            # =============== conv1 phase ===============
            with tc.tile_pool(name="conv1p", bufs=1) as vp, \
                 tc.tile_pool(name="psC", bufs=1, space="PSUM") as ps2:

                # ---------- B1: forward transform of h16 ----------
                V1 = [[vp.tile([128, NSC * U1B], FP16, tag=f"v1_{it}_{dc}",
                               name=f"v1_{it}_{dc}")
                       for dc in range(NDC)] for it in range(NIT)]
                for it in range(NIT):
                    for dc in range(NDC):
                        dsl = slice(dc * 128, (dc + 1) * 128)
                        for sc in range(NSC):
                            pv = ps2.tile([128, U1B + 2 * NT1], F32,
                                          tag="pb2", bufs=2)
                            nmm = 1 + (sc > 0) + (sc < NSC - 1)
                            nc.tensor.matmul(pv[:, 0:U1B],
                                             h16[it][sc][:, dsl],
                                             t_bm1[:, 0:U1B],
                                             start=True, stop=(nmm == 1))
                            i = 1
                            if sc > 0:
                                i += 1
                                nc.tensor.matmul(
                                    pv[:, U1B:U1B + NT1],
                                    h16[it][sc - 1][:, dsl],
                                    t_bm1[:, U1B:U1B + NT1],
                                    start=False, stop=(i == nmm))
                            if sc < NSC - 1:
                                i += 1
                                nc.tensor.matmul(
                                    pv[:, U1B + NT1:U1B + 2 * NT1],
                                    h16[it][sc + 1][:, dsl],
                                    t_bm1[:, U1B + NT1:U1B + 2 * NT1],
                                    start=False, stop=(i == nmm))
                            vdst = V1[it][dc][:].rearrange(
                                "p (t s) -> p t s", t=NT1, s=128)
                            pmain = pv[:, 0:U1B].rearrange(
                                "p (t tl) -> p t tl", t=NT1, tl=16)
                            nc.vector.tensor_copy(
                                vdst[:, :, sc * 16:(sc + 1) * 16], pmain)
                            if sc > 0:
                                nc.vector.tensor_tensor(
                                    vdst[:, :, sc * 16:sc * 16 + 1],
                                    vdst[:, :, sc * 16:sc * 16 + 1],
                                    pv[:, U1B:U1B + NT1].rearrange(
                                        "p (t o) -> p t o", t=NT1, o=1),
                                    ALU.add)
                            if sc < NSC - 1:
                                nc.vector.tensor_tensor(
                                    vdst[:, :, sc * 16 + 15:sc * 16 + 16],
                                    vdst[:, :, sc * 16 + 15:sc * 16 + 16],
                                    pv[:, U1B + NT1:U1B + 2 * NT1].rearrange(
                                        "p (t o) -> p t o", t=NT1, o=1),
                                    ALU.add)

                # ---------- pw1 + Karatsuba-combine + regroup + A1 ------
                # slots grouped per bin; P1,P2,P3 combined to (Mr,Mi) on
                # DVE before the DRAM trip -> M1 has 16 rows, A1 = t_a2.
                # DRAM layout: [cog, it, tau, m*512+c] so stores pair rows
                # and loads regroup with one 3D-AP DMA per (it, cog, b).
                d_m1t = dp.tile([4, NIT, 128, NT2 * 512], FP16, tag="m1d",
                                name="m1d")
                # (slot list, m rows) per bin-group
                bingrp = [([0], [0]), ([1], [1])]
                for f in range(1, 8):
                    b1 = 2 + 3 * (f - 1)
                    b2 = 2 + 2 * (f - 1)
                    bingrp.append(([b1, b1 + 1, b1 + 2], [b2, b2 + 1]))

                def _pw1_mms(pp, it, t, ut, with_bias, cog):
                    for dc in range(NDC):
                        lhs = V1[it][dc][:, t * 128:(t + 1) * 128]
                        nc.tensor.matmul(
                            pp[:], lhs, ut[:, dc * 512:(dc + 1) * 512],
                            start=(dc == 0),
                            stop=(dc == NDC - 1 and not with_bias))
                    if with_bias:
                        nc.tensor.matmul(
                            pp[:], t_ones[0:1, :],
                            t_bc1r[0:1, cog * 512:(cog + 1) * 512],
                            start=False, stop=True)

                for cog in range(4):
                    for slots, mrows in bingrp:
                        uts = []
                        for t in slots:
                            ut = vp.tile([128, NDC * 512], FP16, tag="u1s",
                                         bufs=12, name="u1t")
                            nc.sync.dma_start(ut[:], d_u1[cog, t])
                            uts.append(ut)
                        for it in range(NIT):
                            pps = []
                            for k, t in enumerate(slots):
                                pp = ps2.tile([128, 512], F32, tag="ppw",
                                              bufs=4)
                                _pw1_mms(pp, it, t, uts[k],
                                         t == 0, cog)
                                pps.append(pp)
                            if len(slots) == 1:
                                mg = vp.tile([128, 512], FP16, tag="m1g",
                                             bufs=4, name="mg")
                                nc.vector.tensor_copy(mg[:], pps[0][:])
                                mo = mrows[0] * 512
                                nc.scalar.dma_start(
                                    d_m1t[cog, it][:, mo:mo + 512], mg[:])
                            else:
                                p1, p2, p3 = pps
                                p2s = vp.tile([128, 512], F32, tag="p2s",
                                              bufs=2, name="p2s")
                                nc.vector.tensor_copy(p2s[:], p2[:])
                                mp = vp.tile([128, 1024], FP16, tag="m1p",
                                             bufs=4, name="mp")
                                nc.vector.tensor_tensor(mp[:, 0:512], p1[:],
                                                        p2s[:], ALU.subtract)
                                tmp = vp.tile([128, 512], F32, tag="mtmp",
                                              bufs=2, name="mtmp")
                                nc.vector.tensor_tensor(tmp[:], p3[:],
                                                        p2s[:], ALU.subtract)
                                nc.vector.tensor_tensor(mp[:, 512:1024],
                                                        tmp[:], p1[:],
                                                        ALU.subtract)
                                mo = mrows[0] * 512
                                nc.scalar.dma_start(
                                    d_m1t[cog, it][:, mo:mo + 1024], mp[:])
                    for it in range(NIT):
                        for b in range(16):
                            mh0 = vp.tile([128, 512], FP16, tag="mh0",
                                          bufs=4)
                            nc.scalar.dma_start(
                                mh0[:],
                                d_m1t[cog, it][b * 8:(b + 1) * 8, :]
                                .rearrange("u (m c) -> m u c",
                                           m=NT2, c=512))
                            pa = ps2.tile([64, 512], F32, tag="pa", bufs=2)
                            nc.tensor.matmul(pa[:], t_a2[:, :], mh0[:, :],
                                             start=True, stop=True)
                            c1s = vp.tile([64, 512], FP16, tag="c1s", bufs=4)
                            nc.scalar.activation(c1s[:], pa[:], AF.Relu)
                            ro = (b % 2) * 64
                            nc.gpsimd.dma_start(
                                c1_dram[it][b // 2][ro:ro + 64,
                                                    cog * 512:
                                                    (cog + 1) * 512],
                                c1s[:])

            # =============== conv2 phase ===============
            with tc.tile_pool(name="conv2p", bufs=1) as vp, \
                 tc.tile_pool(name="psD", bufs=1, space="PSUM") as ps2:

                d_m2t = dp.tile([NIT, 128, NT2 * 512], FP16, tag="m2d",
                                name="m2d")

                # ---------- B2: forward transform of c1 ----------
                V2 = [[vp.tile([128, NSC * U2B], FP16, tag=f"v2_{it}_{ch}",
                               name=f"v2_{it}_{ch}")
                       for ch in range(NCD)] for it in range(NIT)]
                for it in range(NIT):
                    c1w = {}

                    def fetch(sc, it=it, c1w=c1w):
                        w = vp.tile([128, CD], FP16, tag="c1w", bufs=4,
                                    name="c1w")
                        nc.sync.dma_start(w[:], c1_dram[it][sc][:])
                        c1w[sc] = w

                    def proc(sc, it=it, c1w=c1w):
                        for ch in range(NCD):
                            csl = slice(ch * 128, (ch + 1) * 128)
                            pv = ps2.tile([128, U2B + 2 * NT2], F32,
                                          tag="pb2", bufs=2)
                            nmm = 1 + (sc > 0) + (sc < NSC - 1)
                            nc.tensor.matmul(pv[:, 0:U2B], c1w[sc][:, csl],
                                             t_bm2[:, 0:U2B],
                                             start=True, stop=(nmm == 1))
                            i = 1
                            if sc > 0:
                                i += 1
                                nc.tensor.matmul(
                                    pv[:, U2B:U2B + NT2],
                                    c1w[sc - 1][:, csl],
                                    t_bm2[:, U2B:U2B + NT2],
                                    start=False, stop=(i == nmm))
                            if sc < NSC - 1:
                                i += 1
                                nc.tensor.matmul(
                                    pv[:, U2B + NT2:U2B + 2 * NT2],
                                    c1w[sc + 1][:, csl],
                                    t_bm2[:, U2B + NT2:U2B + 2 * NT2],
                                    start=False, stop=(i == nmm))
                            vdst = V2[it][ch][:].rearrange(
                                "p (t s) -> p t s", t=NT2, s=128)
                            pmain = pv[:, 0:U2B].rearrange(
                                "p (t tl) -> p t tl", t=NT2, tl=16)
                            nc.vector.tensor_copy(
                                vdst[:, :, sc * 16:(sc + 1) * 16], pmain)
                            if sc > 0:
                                nc.vector.tensor_tensor(
                                    vdst[:, :, sc * 16:sc * 16 + 1],
                                    vdst[:, :, sc * 16:sc * 16 + 1],
                                    pv[:, U2B:U2B + NT2].rearrange(
                                        "p (t o) -> p t o", t=NT2, o=1),
                                    ALU.add)
                            if sc < NSC - 1:
                                nc.vector.tensor_tensor(
                                    vdst[:, :, sc * 16 + 15:sc * 16 + 16],
                                    vdst[:, :, sc * 16 + 15:sc * 16 + 16],
                                    pv[:, U2B + NT2:U2B + 2 * NT2].rearrange(
                                        "p (t o) -> p t o", t=NT2, o=1),
                                    ALU.add)

                    fetch(0)
                    fetch(1)
                    proc(0)
                    for sc in range(1, NSC - 1):
                        fetch(sc + 1)
                        proc(sc)
                    proc(NSC - 1)

                # ---------- pw2 ----------
                mp2 = {}
                for m in range(NT2):
                    slots = [i for i, sl in enumerate(SL2) if sl[4] == m]
                    pps = [ps2.tile([128, 512], F32, tag="ppw", bufs=4,
                                    name=f"pp2{it}")
                           for it in range(NIT)]
                    for k, si in enumerate(slots):
                        vrow = SL2[si][3]
                        for g in range(4):
                            ut = vp.tile([128, 4 * 512], FP16, tag="u2s",
                                         bufs=6, name="u2t")
                            nc.sync.dma_start(ut[:], d_u2[si, g])
                            for c4 in range(4):
                                ch = g * 4 + c4
                                for it in range(NIT):
                                    lhs = V2[it][ch][:,
                                        vrow * 128:(vrow + 1) * 128]
                                    islast = (k == len(slots) - 1 and
                                              ch == NCD - 1 and m != 0)
                                    nc.tensor.matmul(
                                        pps[it][:], lhs,
                                        ut[:, c4 * 512:(c4 + 1) * 512],
                                        start=(k == 0 and ch == 0),
                                        stop=islast)
                    if m == 0:
                        for it in range(NIT):
                            nc.tensor.matmul(pps[it][:], t_ones[0:1, :],
                                             t_bc2r[0:1, :],
                                             start=False, stop=True)
                    for it in range(NIT):
                        if m % 2 == 0:
                            mp2[it] = vp.tile([128, 1024], FP16, tag="m2s",
                                              bufs=4, name="mg2")
                            nc.vector.tensor_copy(mp2[it][:, 0:512],
                                                  pps[it][:])
                        else:
                            nc.vector.tensor_copy(mp2[it][:, 512:1024],
                                                  pps[it][:])
                            mo = (m - 1) * 512
                            nc.scalar.dma_start(
                                d_m2t[it][:, mo:mo + 1024], mp2[it][:])

            # ---------- regroup2 + A2 + ReLU + residual + LN2 -------
            with tc.tile_pool(name="conv2b", bufs=1) as vp, \
                 tc.tile_pool(name="psE", bufs=1, space="PSUM") as ps2:
                for sc in range(NSC):
                    for it in range(NIT):
                        eng = nc.sync if it == 0 else nc.scalar
                        pa2 = ps2.tile([128, 512], F32, tag="pa", bufs=2)
                        for half in range(2):
                            b = 2 * sc + half
                            mh2 = vp.tile([128, 512], FP16, tag="mh2",
                                          bufs=4)
                            eng.dma_start(
                                mh2[:],
                                d_m2t[it][b * 8:(b + 1) * 8, :]
                                .rearrange("u (m c) -> m u c",
                                           m=NT2, c=512))
                            ro = half * 64
                            nc.tensor.matmul(pa2[ro:ro + 64, :], t_a2[:, :],
                                             mh2[:, :], start=True,
                                             stop=True)
                        t1 = vp.tile([128, 512], F32, tag="t1", bufs=3)
                        nc.scalar.activation(t1[:], pa2[:], AF.Relu)
                        r = vp.tile([128, 512], F32, tag="r2", bufs=4)
                        nc.vector.tensor_tensor(r[:], t1[:], h16[it][sc][:],
                                                ALU.add)
                        st6 = vp.tile([128, 6], F32, tag="st62", bufs=2)
                        mv = vp.tile([128, 2], F32, tag="mv2", bufs=2)
                        nc.vector.bn_stats(st6[:], r[:])
                        nc.vector.bn_aggr(mv[:], st6[:])
                        inv = vp.tile([128, 1], F32, tag="inv2", bufs=2)
                        nc.scalar.activation(inv[:], mv[:, 1:2], AF.Sqrt,
                                             bias=t_eps[:])
                        nc.vector.reciprocal(inv[:], inv[:])
                        yt = vp.tile([128, 512], F32, tag="yt2", bufs=3)
                        nc.vector.tensor_scalar(yt[:], r[:], mv[:, 0:1],
                                                inv[:], ALU.subtract,
                                                ALU.mult)
                        if affine2:
                            nc.vector.tensor_tensor(yt[:], yt[:],
                                                    t_gb[G2][:], ALU.mult)
                            nc.vector.tensor_tensor(yt[:], yt[:],
                                                    t_gb[B2b][:], ALU.add)
                        nc.sync.dma_start(d_y[it, sc], yt[:])

    nc.compile()
    _BUILT[key] = nc
    return nc


def _prep_host(inputs):
    x = np.asarray(inputs["x"], np.float32)
    Wq = np.asarray(inputs["Wq"], np.float32)
    bq = np.asarray(inputs["bq"], np.float32)
    Wk = np.asarray(inputs["Wk"], np.float32)
    bk = np.asarray(inputs["bk"], np.float32)
    Wv = np.asarray(inputs["Wv"], np.float32)
    bv = np.asarray(inputs["bv"], np.float32)
    Wo = np.asarray(inputs["Wo"], np.float32)
    bo = np.asarray(inputs["bo"], np.float32)
    g1 = np.asarray(inputs["g1"], np.float32)
    b1 = np.asarray(inputs["b1"], np.float32)
    g2 = np.asarray(inputs["g2"], np.float32)
    b2 = np.asarray(inputs["b2"], np.float32)
    Wc1 = np.asarray(inputs["Wc1"], np.float64)
    bc1 = np.asarray(inputs["bc1"], np.float32)
    Wc2 = np.asarray(inputs["Wc2"], np.float64)
    bc2 = np.asarray(inputs["bc2"], np.float32)

    xT = np.ascontiguousarray(x.transpose(0, 2, 1).reshape(B, NDC, 128, S))
    xp = np.ascontiguousarray((x + bo[None, None, :]).reshape(B, NSC, 128, D))

    wqk = np.zeros((2, 4, 128, 512), np.float32)
    for proj, W in ((0, Wq), (1, Wk)):
        for pair in range(4):
            blk = np.concatenate([W[2 * pair], W[2 * pair + 1]], axis=1)
            wqk[proj, pair] = blk.reshape(NDC, 128, 128).transpose(1, 0, 2) \
                                 .reshape(128, 512)
    bqk = np.zeros((128, 8), np.float32)
    for proj, b in ((0, bq), (1, bk)):
        for pair in range(4):
            bqk[:, proj * 4 + pair] = np.concatenate(
                [b[2 * pair], b[2 * pair + 1]])

    wv = np.zeros((NDC, 128, 520), np.float32)
    bvrow = np.zeros((128, 520), np.float32)
    for h in range(H):
        wv[:, :, h * 65:h * 65 + 64] = Wv[h].reshape(NDC, 128, 64)
        bvrow[:, h * 65:h * 65 + 64] = bv[h][None, :]
        bvrow[:, h * 65 + 64] = 1.0

    wo = np.ascontiguousarray(Wo.reshape(4, 128, 512))

    # ---- conv transform constants ----
    BT1, AT1, BT2, AT2, SL2 = _dft_mats()
    bm1 = _bmat(BT1).astype(np.float16)
    bm2 = _bmat(BT2).astype(np.float16)
    a0 = _amat_packed(AT1, list(range(12))).astype(np.float16)
    a1 = _amat_packed(AT1, list(range(12, NT1))).astype(np.float16)
    a2 = _amat_packed(AT2, list(range(NT2))).astype(np.float16)
    ones = np.ones((1, 128), np.float16)
    bc1r = bc1.reshape(1, CD).astype(np.float16)
    bc2r = bc2.reshape(1, D).astype(np.float16)

    U1 = _u_slots1(Wc1)                              # [23, 512, 2048]
    u1 = np.ascontiguousarray(
        U1.reshape(NT1, NDC, 128, 4, 512).transpose(3, 0, 2, 1, 4)
          .reshape(4, NT1, 128, NDC * 512)).astype(np.float16)
    U2 = _u_slots2(Wc2, SL2)                         # [30, 2048, 512]
    u2 = np.ascontiguousarray(
        U2.reshape(NSL2, 4, 4, 128, 512).transpose(0, 1, 3, 2, 4)
          .reshape(NSL2, 4, 128, 4 * 512)).astype(np.float16)

    gb = np.stack([np.tile(v[None, :], (128, 1))
                   for v in (g1, b1, g2, b2)]).astype(np.float32)
    cones = np.ones((128, 128), np.float32)

    shared = dict(wqk=wqk, bqk=bqk, wv=wv, bvrow=bvrow, wo=wo, gb=gb,
                  cones=cones, bm1=bm1, bm2=bm2, a0=a0, a1=a1, a2=a2,
                  ones=ones, bc1r=bc1r, bc2r=bc2r, u1=u1, u2=u2)
    in_maps = []
    for c in range(NCORES):
        m = dict(shared)
        m["xT"] = np.ascontiguousarray(xT[c * NIT:(c + 1) * NIT])
        m["xp"] = np.ascontiguousarray(xp[c * NIT:(c + 1) * NIT])
        in_maps.append(m)
    return in_maps


def run(inputs, trace=False, **trace_kwargs):
    affine1 = not (np.all(np.asarray(inputs["g1"]) == 1.0)
                   and np.all(np.asarray(inputs["b1"]) == 0.0))
    affine2 = not (np.all(np.asarray(inputs["g2"]) == 1.0)
                   and np.all(np.asarray(inputs["b2"]) == 0.0))
    nc = _build(affine1, affine2)
    from concourse.bass_utils import run_bass_kernel_spmd
    in_maps = _prep_host(inputs)
    res = run_bass_kernel_spmd(nc, in_maps, core_ids=list(range(NCORES)),
                               trace=trace, **trace_kwargs)
    y = np.concatenate([res.results[c]["y"].reshape(NIT, S, D)
                        for c in range(NCORES)], axis=0)
    return y, res


def kernel(**inputs):
    y, _ = run(inputs, trace=False)
    return y
